# revision 2
# baseline (speedup 1.0000x reference)
"""STFT (n_fft=4096, hop=1024, centered reflect-pad, Hann) on 8 TRN2 cores.

Algorithm: 2-stage Cooley-Tukey, n = 128*n1 + n2 (n1 in [0,32), n2 in [0,128)),
k = k1 + 32*k2 (k1 in [0,32), k2 in [0,64] for the 2049 kept bins).

  X[k1+32k2, b] = sum_n2 G[n2,k] * sum_n1 e^{-2pi i n1 k1/32} * xw[b, 128n1+n2]

Stage 1 runs frames-as-weights so its output lands transposed (n2 on
partitions): per 4-frame subgroup one [128,128] lhsT (4 frames interleaved
across partitions) against a constant one-hot-structured rhs [128,256].
Stage 2 contracts n2 (K=128) with per-k1 twiddle matrices in fp16.

Windowing is folded into host-side input prep: 4 phase-shifted windowed
copies of the padded signal (xw_j = xp * w[1024j + p mod 1024]); the framing
DMA reads each frame quarter from the matching copy.

Partition mapping of stage-1 lhsT rows: p = 32*j + 4*i + r  (j = quarter,
i = n1 mod 8 ... n1 = 8j+i, r = frame-in-subgroup). Each (plane, j) framing
DMA then fills 32 contiguous partitions in one shot.

Sharding: frame-parallel. Core i computes 516 frames starting at frame 512*i
(SPMD, same NEFF); host trims/concatenates to the 4097 global frames.
"""

import numpy as np

import concourse.bacc as bacc
import concourse.tile as tile
import concourse.mybir as mybir
from concourse import bass_utils

N_FFT = 4096
HOP = 1024
T = 4194304
NBINS = N_FFT // 2 + 1          # 2049
F_TOTAL = T // HOP + 1          # 4097
NCORES = 8

NF = 516                        # frames computed per core (129 subgroups of 4)
GROUPS = [128, 128, 128, 128, 4]
L = (NF - 1) * HOP + N_FFT      # per-core input samples per plane = 531456
XW_LEN = 2 * L + 8192           # flat [plane0 | plane1 | slack] per xw tensor

F32R = mybir.dt.float32r
F32 = mybir.dt.float32
F16 = mybir.dt.float16

_cache = {}


def _host_constants():
    n1 = np.arange(32)
    k1 = np.arange(32)
    C = np.cos(2 * np.pi * np.outer(n1, k1) / 32)
    S = np.sin(2 * np.pi * np.outer(n1, k1) / 32)
    R1 = np.concatenate([C, -S], axis=1)      # [n1, 64]
    R2 = np.concatenate([S, C], axis=1)
    # lhsT partition p = 32j + 8r + i  <->  (n1 = 8j+i, frame r)
    R1D = np.zeros((128, 256), np.float32)
    R2D = np.zeros((128, 256), np.float32)
    for j in range(4):
        for i in range(8):
            for r in range(4):
                p = 32 * j + 8 * r + i
                R1D[p, 64 * r:64 * r + 64] = R1[8 * j + i]
                R2D[p, 64 * r:64 * r + 64] = R2[8 * j + i]

    n2 = np.arange(128)
    k2 = np.arange(64)
    Gp = np.zeros((128, 32 * 128), np.float16)
    Gq = np.zeros((128, 32 * 128), np.float16)
    for q in range(32):
        kk = q + 32 * k2
        ang = 2 * np.pi * np.outer(n2, kk) / N_FFT
        gr = np.cos(ang)
        gi = -np.sin(ang)
        Gp[:, 128 * q:128 * q + 64] = gr.astype(np.float16)
        Gp[:, 128 * q + 64:128 * q + 128] = gi.astype(np.float16)
        Gq[:, 128 * q:128 * q + 64] = (-gi).astype(np.float16)
        Gq[:, 128 * q + 64:128 * q + 128] = gr.astype(np.float16)

    alt = ((-1.0) ** n2).astype(np.float16)
    E1 = np.zeros((128, 2), np.float16)
    E2 = np.zeros((128, 2), np.float16)
    E1[:, 0] = alt
    E2[:, 1] = alt
    return (R1D, R2D, Gp, Gq, E1, E2)


def _build(stages=("dma", "s1", "s2", "out")):
    stages = set(stages)
    nc = bacc.Bacc("TRN2", target_bir_lowering=False, debug=False,
                   enable_asserts=False, num_devices=NCORES)
    xw = [nc.dram_tensor(f"xw{j}", [XW_LEN], F32R, kind="ExternalInput")
          for j in range(4)]
    r1d = nc.dram_tensor("r1d", [128, 256], F32R, kind="ExternalInput")
    r2d = nc.dram_tensor("r2d", [128, 256], F32R, kind="ExternalInput")
    gp = nc.dram_tensor("gp", [128, 32 * 128], F16, kind="ExternalInput")
    gq = nc.dram_tensor("gq", [128, 32 * 128], F16, kind="ExternalInput")
    e1 = nc.dram_tensor("e1", [128, 2], F16, kind="ExternalInput")
    e2 = nc.dram_tensor("e2", [128, 2], F16, kind="ExternalInput")
    out = nc.dram_tensor("o", [2, 2048, NF], F32, kind="ExternalOutput")
    oute = nc.dram_tensor("oe", [2, 1, NF], F32, kind="ExternalOutput")

    with tile.TileContext(nc) as tc:
        with (
            tc.tile_pool(name="const", bufs=1) as cpool,
            tc.tile_pool(name="fr", bufs=2) as frpool,
            tc.tile_pool(name="ys", bufs=2) as yspool,
            tc.tile_pool(name="ost", bufs=2) as ostpool,
            tc.tile_pool(name="ps1", bufs=3, space="PSUM") as ps1pool,
            tc.tile_pool(name="ps2", bufs=3, space="PSUM") as ps2pool,
            tc.tile_pool(name="pse", bufs=2, space="PSUM") as psepool,
        ):
            t_r1 = cpool.tile([128, 256], F32R, tag="r1")
            t_r2 = cpool.tile([128, 256], F32R, tag="r2")
            t_gp = cpool.tile([128, 32 * 128], F16, tag="gp")
            t_gq = cpool.tile([128, 32 * 128], F16, tag="gq")
            t_e1 = cpool.tile([128, 2], F16, tag="e1")
            t_e2 = cpool.tile([128, 2], F16, tag="e2")
            nc.sync.dma_start(t_r1[:], r1d.ap()[:, :])
            nc.sync.dma_start(t_r2[:], r2d.ap()[:, :])
            nc.sync.dma_start(t_gp[:], gp.ap()[:, :])
            nc.sync.dma_start(t_gq[:], gq.ap()[:, :])
            nc.sync.dma_start(t_e1[:], e1.ap()[:, :])
            nc.sync.dma_start(t_e2[:], e2.ap()[:, :])

            gb0 = 0
            group_state = []          # (gb0, B, ys) awaiting stage-2
            starts = []
            for B in GROUPS:
                starts.append(gb0)
                gb0 += B

            def emit_load_s1(gb0, B):
                nsub = B // 4
                ncols = 128 * nsub
                fr_r = frpool.tile([128, 128 * 32], F32R, tag="fr_r")
                fr_i = frpool.tile([128, 128 * 32], F32R, tag="fr_i")
                # framing DMA: FR[32j+8r+i, 128s+m] =
                #   xw_j[c*L + 1024*(gb0+4s+r) + 1024j + 128i + m]
                # = dense read of xw_j[off0 : off0+4096*nsub] as (s p m)
                for c, fr_t in ((0, fr_r), (1, fr_i)):
                    for j in range(4):
                        off0 = c * L + HOP * gb0 + 1024 * j
                        seg = xw[j].ap()[off0:off0 + 4096 * nsub]
                        srca = seg.rearrange("(s p m) -> p s m", p=32, m=128)
                        dst = fr_t[32 * j:32 * j + 32, 0:ncols]
                        dst = dst.rearrange("p (s m) -> p s m", m=128)
                        nc.sync.dma_start(dst, srca)

                ys = yspool.tile([128, 64 * 128], F16, tag="ys")
                if "s1" not in stages:
                    return ys
                npair = (nsub + 1) // 2
                for sp in range(npair):
                    s0 = 2 * sp
                    nsg = min(2, nsub - s0)
                    w = 256 * nsg
                    ps1 = ps1pool.tile([128, 512], F32, tag="ps1")
                    for t in range(nsg):
                        s = s0 + t
                        cs = 256 * t
                        nc.tensor.matmul(ps1[:, cs:cs + 256],
                                         fr_r[:, 128 * s:128 * s + 128],
                                         t_r1[:], start=(t == 0), stop=False)
                        nc.tensor.matmul(ps1[:, cs:cs + 256],
                                         fr_i[:, 128 * s:128 * s + 128],
                                         t_r2[:], start=False,
                                         stop=(t == nsg - 1))
                    dstc = ys[:, 256 * s0:256 * s0 + w]
                    if sp % 2 == 0:
                        nc.vector.tensor_copy(dstc, ps1[:, 0:w])
                    else:
                        nc.scalar.copy(dstc, ps1[:, 0:w])
                return ys

            def emit_s2_out(gb0, B, ys):
                if "s2" not in stages:
                    return
                ysv = ys[:, 0:64 * B].rearrange("p (b j) -> p j b", j=64)
                ost = ostpool.tile([128, 32 * 128], F32, tag="ost")
                for qp in range(16):
                    q0 = 2 * qp
                    ps2 = ps2pool.tile([128, 256], F32, tag="ps2")
                    for t in range(2):
                        q = q0 + t
                        rhs_r = ysv[:, q:q + 1, :].rearrange(
                            "p o b -> p (o b)")
                        rhs_i = ysv[:, 32 + q:33 + q, :].rearrange(
                            "p o b -> p (o b)")
                        cs = 128 * t
                        nc.tensor.matmul(ps2[:, cs:cs + B],
                                         t_gp[:, 128 * q:128 * q + 128],
                                         rhs_r, start=(t == 0), stop=False)
                        nc.tensor.matmul(ps2[:, cs:cs + B],
                                         t_gq[:, 128 * q:128 * q + 128],
                                         rhs_i, start=False, stop=(t == 1))
                    d0 = ost[:, 128 * q0:128 * q0 + B]
                    d1 = ost[:, 128 * (q0 + 1):128 * (q0 + 1) + B]
                    if qp % 2 == 0:
                        nc.vector.tensor_copy(d0, ps2[:, 0:B])
                        nc.vector.tensor_copy(d1, ps2[:, 128:128 + B])
                    else:
                        nc.scalar.copy(d0, ps2[:, 0:B])
                        nc.scalar.copy(d1, ps2[:, 128:128 + B])
                    if "out" in stages and qp == 7:
                        srcp = ost[:, 0:16 * 128].rearrange(
                            "p (q b) -> p q b", b=128)[:, :, 0:B]
                        dst = out.ap()[:, :, gb0:gb0 + B]
                        dst = dst.rearrange(
                            "c (p q) b -> (c p) q b", q=32)[:, 0:16, :]
                        nc.scalar.dma_start(dst, srcp)

                # bin 2048 (k1=0, k2=64)
                pse = psepool.tile([2, 128], F32, tag="pse")
                rhs_r0 = ysv[:, 0:1, :].rearrange("p o b -> p (o b)")
                rhs_i0 = ysv[:, 32:33, :].rearrange("p o b -> p (o b)")
                nc.tensor.matmul(pse[:, 0:B], t_e1[:], rhs_r0,
                                 start=True, stop=False)
                nc.tensor.matmul(pse[:, 0:B], t_e2[:], rhs_i0,
                                 start=False, stop=True)
                oste = ostpool.tile([2, 128], F32, tag="oste")
                nc.vector.tensor_copy(oste[:, 0:B], pse[:, 0:B])

                if "out" in stages:
                    srcp = ost[:, 16 * 128:].rearrange(
                        "p (q b) -> p q b", b=128)[:, :, 0:B]
                    dst = out.ap()[:, :, gb0:gb0 + B]
                    dst = dst.rearrange(
                        "c (p q) b -> (c p) q b", q=32)[:, 16:32, :]
                    nc.scalar.dma_start(dst, srcp)
                    dste = oute.ap()[:, 0, gb0:gb0 + B]
                    nc.scalar.dma_start(dste, oste[:, 0:B])

            pending = None
            for gi, B in enumerate(GROUPS):
                ys = emit_load_s1(starts[gi], B)
                if pending is not None:
                    emit_s2_out(*pending)
                pending = (starts[gi], B, ys)
            emit_s2_out(*pending)

    nc.compile()
    return nc


def _prep_inputs(x, window):
    pad = N_FFT // 2
    xp = np.pad(np.asarray(x), ((0, 0), (pad, pad)), mode="reflect")
    total = xp.shape[1]
    need = (NCORES - 1) * 512 * HOP + L
    xp_ext = np.zeros((2, max(total, need)), np.float32)
    xp_ext[:, :total] = xp
    w = np.asarray(window, np.float32)
    reps = xp_ext.shape[1] // HOP + 1
    xw_full = []
    for j in range(4):
        wj = np.tile(w[HOP * j:HOP * (j + 1)], reps)[:xp_ext.shape[1]]
        xw_full.append(xp_ext * wj[None, :])
    return xw_full


def kernel(x, window):
    import time
    t0 = time.time()
    x = np.asarray(x, np.float32)
    window = np.asarray(window, np.float32)
    if "nc" not in _cache:
        _cache["nc"] = _build()
    nc = _cache["nc"]
    print(f"[kernel] build done {time.time()-t0:.2f}s", flush=True)

    xw_full = _prep_inputs(x, window)
    R1D, R2D, Gp, Gq, E1, E2 = _host_constants()

    in_maps = []
    for i in range(NCORES):
        s0 = i * 512 * HOP
        m = {"r1d": R1D, "r2d": R2D, "gp": Gp, "gq": Gq, "e1": E1, "e2": E2}
        for j in range(4):
            flat = np.zeros(XW_LEN, np.float32)
            flat[:L] = xw_full[j][0, s0:s0 + L]
            flat[L:2 * L] = xw_full[j][1, s0:s0 + L]
            m[f"xw{j}"] = flat
        in_maps.append(m)

    print(f"[kernel] inputs prepped {time.time()-t0:.2f}s", flush=True)
    import os
    trace = bool(os.environ.get("KERNEL_TRACE"))
    res = bass_utils.run_bass_kernel_spmd(nc, in_maps,
                                          core_ids=list(range(NCORES)),
                                          trace=trace)
    if trace and res.exec_time_ns is not None:
        global LAST_EXEC_NS
        LAST_EXEC_NS = res.exec_time_ns
        print(f"[kernel] exec_time_ns={res.exec_time_ns}", flush=True)
        if res.instructions_and_trace is not None:
            print(f"[kernel] trace: {res.instructions_and_trace[1]}",
                  flush=True)
    print(f"[kernel] spmd done {time.time()-t0:.2f}s", flush=True)
    out = np.zeros((2, NBINS, F_TOTAL), np.float32)
    for i in range(NCORES):
        o = res.results[i]["o"]
        oe = res.results[i]["oe"]
        f0 = 512 * i
        nf = 513 if i == NCORES - 1 else 512
        out[:, :2048, f0:f0 + nf] = o[:, :, :nf]
        out[:, 2048, f0:f0 + nf] = oe[:, 0, :nf]
    return out



# revision 4
# speedup vs baseline: 2.1943x; 2.1943x over previous
"""STFT (n_fft=4096, hop=1024, centered reflect-pad, windowed) on 8 TRN2 cores.

Algorithm: 2-stage Cooley-Tukey, n = 128*n1 + n2 (n1 in [0,32), n2 in [0,128)),
k = k1 + 32*k2 (k1 in [0,32), k2 in [0,64] for the 2049 kept bins).

Stage 1 packs BOTH complex planes and a 2-frame subgroup into the matmul
contraction: K = (plane, j, i, r) with n1 = 8j + i, r = frame parity.
lhsT = windowed frame data (stationary), rhs = a constant [128,128]
twiddle R12 -> psum [n2, (k1, comp, r)] in a single non-accumulating
matmul per subgroup (64 PE cycles/frame).

Host-side prep writes the frame data ALREADY in stage-1 lhsT layout as
fp16 ("frin"), so the input DMA is fully dense: 128 descriptors x 8KB
per 32-subgroup chunk (vs 512B gather packets when framing on-device).

Stage 2 runs q(=k1)-outer over ALL 516 frames at once with fp16 twiddle
blocks Gp/Gq reused across frame chunks; outputs accumulate in SBUF as
[128, 516] rows and DMA out with 2064B descriptors (f contiguous).

Partition map of stage-1 lhsT rows: p = 64*pl + 16*j + 2*i + r.
frin[p, 128*s + m] = xw_j[pl, 1024*(2s + r + j) + 128*i + m] where xw_j
is the j-th phase-windowed padded signal (built host-side, fp16).

Sharding: frame-parallel. Core i computes 516 frames starting at frame
512*i (SPMD, same NEFF); host trims/concatenates to 4097 global frames.
"""

import numpy as np

import concourse.bacc as bacc
import concourse.tile as tile
import concourse.mybir as mybir
from concourse import bass_utils

N_FFT = 4096
HOP = 1024
T = 4194304
NBINS = N_FFT // 2 + 1          # 2049
F_TOTAL = T // HOP + 1          # 4097
NCORES = 8

NF = 516                        # frames per core
NS = NF // 2                    # 258 subgroups of 2 frames
L = (NF - 1) * HOP + N_FFT      # per-core span of samples = 531456
SGROUPS = [32] * 8 + [2]        # subgroup chunks per input DMA
FCHUNKS = ((0, 129), (129, 258))  # stage-2 subgroup (frame/2) chunks

F32 = mybir.dt.float32
F16 = mybir.dt.float16

_cache = {}


def _host_constants():
    n1g = np.arange(32)
    k1g = np.arange(32)
    C = np.cos(2 * np.pi * np.outer(n1g, k1g) / 32)
    S = np.sin(2 * np.pi * np.outer(n1g, k1g) / 32)
    M = ((C, -S), (S, C))       # M[pl][comp]
    R12 = np.zeros((128, 128), np.float16)
    for pl in range(2):
        for c in range(2):
            blk = M[pl][c]      # [n1, k1]
            for j in range(4):
                for i in range(8):
                    for r in range(2):
                        p = 64 * pl + 16 * j + 2 * i + r
                        R12[p, 4 * k1g + 2 * c + r] = blk[8 * j + i]

    n2 = np.arange(128)
    k2 = np.arange(64)
    Gp = np.zeros((128, 32 * 128), np.float16)
    Gq = np.zeros((128, 32 * 128), np.float16)
    for q in range(32):
        kk = q + 32 * k2
        ang = 2 * np.pi * np.outer(n2, kk) / N_FFT
        gr = np.cos(ang)
        gi = -np.sin(ang)
        Gp[:, 128 * q:128 * q + 64] = gr.astype(np.float16)
        Gp[:, 128 * q + 64:128 * q + 128] = gi.astype(np.float16)
        Gq[:, 128 * q:128 * q + 64] = (-gi).astype(np.float16)
        Gq[:, 128 * q + 64:128 * q + 128] = gr.astype(np.float16)

    alt = ((-1.0) ** n2).astype(np.float16)
    E1 = np.zeros((128, 2), np.float16)
    E2 = np.zeros((128, 2), np.float16)
    E1[:, 0] = alt
    E2[:, 1] = alt
    return (R12, Gp, Gq, E1, E2)


def _build():
    nc = bacc.Bacc("TRN2", target_bir_lowering=False, debug=False,
                   enable_asserts=False, num_devices=NCORES)
    frin = nc.dram_tensor("frin", [128, 128 * NS], F16, kind="ExternalInput")
    r12 = nc.dram_tensor("r12", [128, 128], F16, kind="ExternalInput")
    gp = nc.dram_tensor("gp", [128, 32 * 128], F16, kind="ExternalInput")
    gq = nc.dram_tensor("gq", [128, 32 * 128], F16, kind="ExternalInput")
    e1 = nc.dram_tensor("e1", [128, 2], F16, kind="ExternalInput")
    e2 = nc.dram_tensor("e2", [128, 2], F16, kind="ExternalInput")
    out = nc.dram_tensor("o", [2, 2048, NF], F32, kind="ExternalOutput")
    oute = nc.dram_tensor("oe", [2, 1, NF], F32, kind="ExternalOutput")

    with tile.TileContext(nc) as tc:
        with (
            tc.tile_pool(name="const", bufs=1) as cpool,
            tc.tile_pool(name="fr", bufs=3) as frpool,
            tc.tile_pool(name="ys", bufs=1) as yspool,
            tc.tile_pool(name="ost", bufs=3) as ostpool,
            tc.tile_pool(name="ps1", bufs=3, space="PSUM") as ps1pool,
            tc.tile_pool(name="ps2", bufs=4, space="PSUM") as ps2pool,
            tc.tile_pool(name="pse", bufs=1, space="PSUM") as psepool,
        ):
            t_r12 = cpool.tile([128, 128], F16, tag="r12")
            t_gp = cpool.tile([128, 32 * 128], F16, tag="gp")
            t_gq = cpool.tile([128, 32 * 128], F16, tag="gq")
            t_e1 = cpool.tile([128, 2], F16, tag="e1")
            t_e2 = cpool.tile([128, 2], F16, tag="e2")
            nc.sync.dma_start(t_r12[:], r12.ap()[:, :])
            nc.sync.dma_start(t_gp[:], gp.ap()[:, :])
            nc.sync.dma_start(t_gq[:], gq.ap()[:, :])
            nc.sync.dma_start(t_e1[:], e1.ap()[:, :])
            nc.sync.dma_start(t_e2[:], e2.ap()[:, :])

            t_ys = yspool.tile([128, 128 * NS], F16, tag="ys")

            # ---- Phase A: dense input DMA + stage 1 + psum->ys casts ----
            cp_ix = 0
            s0 = 0
            for ns in SGROUPS:
                fr = frpool.tile([128, 128 * ns], F16, tag="fr")
                nc.sync.dma_start(fr[:],
                                  frin.ap()[:, 128 * s0:128 * (s0 + ns)])
                for b in range(0, ns, 4):
                    nb = min(4, ns - b)
                    ps = ps1pool.tile([128, 512], F32, tag="ps1")
                    for t in range(nb):
                        nc.tensor.matmul(ps[:, 128 * t:128 * t + 128],
                                         fr[:, 128 * (b + t):128 * (b + t + 1)],
                                         t_r12[:], start=True, stop=True)
                    dst = t_ys[:, 128 * (s0 + b):128 * (s0 + b + nb)]
                    if cp_ix % 2 == 0:
                        nc.vector.tensor_copy(dst, ps[:, 0:128 * nb])
                    else:
                        nc.scalar.copy(dst, ps[:, 0:128 * nb])
                    cp_ix += 1
                s0 += ns

            # ---- Phase B: stage 2 (q-outer over all frames) + output ----
            ysv = t_ys[:, :].rearrange("p (s x) -> p s x", x=128)
            for qp in range(16):
                ost = ostpool.tile([128, 2 * NF], F32, tag="ost")
                for t in range(2):
                    q = 2 * qp + t
                    for ci, (sa, sb) in enumerate(FCHUNKS):
                        n = 2 * (sb - sa)
                        ps = ps2pool.tile([128, 258], F32, tag="ps2")
                        rhs_r = ysv[:, sa:sb, 4 * q:4 * q + 2]
                        rhs_i = ysv[:, sa:sb, 4 * q + 2:4 * q + 4]
                        nc.tensor.matmul(ps[:, 0:n],
                                         t_gp[:, 128 * q:128 * q + 128],
                                         rhs_r, start=True, stop=False)
                        nc.tensor.matmul(ps[:, 0:n],
                                         t_gq[:, 128 * q:128 * q + 128],
                                         rhs_i, start=False, stop=True)
                        dst = ost[:, NF * t + 2 * sa:NF * t + 2 * sb]
                        if (2 * t + ci) % 2 == 0:
                            nc.vector.tensor_copy(dst, ps[:, 0:n])
                        else:
                            nc.scalar.copy(dst, ps[:, 0:n])
                dstq = out.ap().rearrange("c (k q) b -> (c k) q b",
                                          q=32)[:, 2 * qp:2 * qp + 2, :]
                srcq = ost[:].rearrange("p (q b) -> p q b", b=NF)
                nc.sync.dma_start(dstq, srcq)

            # bin 2048 (k1=0, k2=64): +/- sum over n2 of Y[0]
            oste = ostpool.tile([2, NF], F32, tag="oste")
            for ci, (sa, sb) in enumerate(FCHUNKS):
                n = 2 * (sb - sa)
                pse = psepool.tile([2, 258], F32, tag="pse")
                rhs_r0 = ysv[:, sa:sb, 0:2]
                rhs_i0 = ysv[:, sa:sb, 2:4]
                nc.tensor.matmul(pse[:, 0:n], t_e1[:], rhs_r0,
                                 start=True, stop=False)
                nc.tensor.matmul(pse[:, 0:n], t_e2[:], rhs_i0,
                                 start=False, stop=True)
                nc.vector.tensor_copy(oste[:, 2 * sa:2 * sb], pse[:, 0:n])
            nc.sync.dma_start(oute.ap()[:, 0, :], oste[:])

    nc.compile()
    return nc


def _prep_inputs(x, window):
    pad = N_FFT // 2
    xp = np.pad(np.asarray(x), ((0, 0), (pad, pad)), mode="reflect")
    total = xp.shape[1]
    need = (NCORES - 1) * 512 * HOP + L + 8192
    xp_ext = np.zeros((2, max(total, need)), np.float32)
    xp_ext[:, :total] = xp
    w = np.asarray(window, np.float32)
    reps = xp_ext.shape[1] // HOP + 1
    xws = []
    for j in range(4):
        wj = np.tile(w[HOP * j:HOP * (j + 1)], reps)[:xp_ext.shape[1]]
        xws.append(xp_ext * wj[None, :])

    frins = []
    for i in range(NCORES):
        s0 = i * 512 * HOP
        fr = np.empty((128, 128 * NS), np.float16)
        for pl in range(2):
            for j in range(4):
                seg = xws[j][pl]
                v = np.lib.stride_tricks.as_strided(
                    seg[s0 + 1024 * j:], (8, 2, NS, 128),
                    (128 * 4, 1024 * 4, 2048 * 4, 4))
                fr[64 * pl + 16 * j:64 * pl + 16 * j + 16] = \
                    v.reshape(16, 128 * NS)
        frins.append(fr)
    return frins


def kernel(x, window):
    import time
    t0 = time.time()
    x = np.asarray(x, np.float32)
    window = np.asarray(window, np.float32)
    if "nc" not in _cache:
        _cache["nc"] = _build()
    nc = _cache["nc"]
    print(f"[kernel] build done {time.time()-t0:.2f}s", flush=True)

    frins = _prep_inputs(x, window)
    R12, Gp, Gq, E1, E2 = _host_constants()

    in_maps = []
    for i in range(NCORES):
        in_maps.append({"frin": frins[i], "r12": R12, "gp": Gp, "gq": Gq,
                        "e1": E1, "e2": E2})

    print(f"[kernel] inputs prepped {time.time()-t0:.2f}s", flush=True)
    import os
    trace = bool(os.environ.get("KERNEL_TRACE"))
    res = bass_utils.run_bass_kernel_spmd(nc, in_maps,
                                          core_ids=list(range(NCORES)),
                                          trace=trace)
    if trace and res.exec_time_ns is not None:
        global LAST_EXEC_NS
        LAST_EXEC_NS = res.exec_time_ns
        print(f"[kernel] exec_time_ns={res.exec_time_ns}", flush=True)
        if res.instructions_and_trace is not None:
            print(f"[kernel] trace: {res.instructions_and_trace[1]}",
                  flush=True)
    print(f"[kernel] spmd done {time.time()-t0:.2f}s", flush=True)
    out = np.zeros((2, NBINS, F_TOTAL), np.float32)
    for i in range(NCORES):
        o = res.results[i]["o"]
        oe = res.results[i]["oe"]
        f0 = 512 * i
        nf = 513 if i == NCORES - 1 else 512
        out[:, :2048, f0:f0 + nf] = o[:, :, :nf]
        out[:, 2048, f0:f0 + nf] = oe[:, 0, :nf]
    return out
